# revision 36
# baseline (speedup 1.0000x reference)
"""GPT-OSS attention QK+softmax block (sliding-window 128, softmax with sink)
for Trainium2, sharded over the 8 kv heads across 8 NeuronCores.

Reference computation (per kv head h, per q-head m):
    S = (q[:, h, m] @ k[:, h].T) / sqrt(64)            # [T, T]
    S += causal & sliding-window(128) mask             # band of width 128
    probs = softmax([S, sink_{h,m}])[..., :-1]         # sink column dropped

Device kernel structure (per core = one kv head), v2 "stacked" layout:
  * the 128 PSUM partitions hold 8 q-heads x 16 queries (p = m*16 + r16),
    so one matmul covers ALL m-heads for a 16-query sub-block s
    (queries 16s..16s+15).  The key window for those queries is only
    16 + 128 = 144 wide (cols = keys 16s-128 .. 16s+16), vs 256 in a
    128-query blocking -- every downstream stage (exp, row-sums,
    normalize, output DMA) shrinks ~40%.
  * 64 sub-blocks, processed in 11 groups of 6 (last group 4).  One
    PSUM tile per group = 2 banks; each bank holds 3 slots of 144
    fp32 cols (+80 pad).
  * matmul cost on the PE scales with OUTPUT cols only, so q enters
    exactly for free: stationary = [q_hi; q_lo] (128 contraction rows),
    moving = [k_bf16; k_bf16].  k is single-rounded (max rel err
    ~1.1e-2 incl fp16 storage, vs the 2e-2 gate).
  * causal/sliding-window mask folded into scores on the PE: identity-
    weight matmul accumulates a {0,-1e4} bias per slot so exp
    underflows masked entries to exactly 0.  Sub-blocks s<8 use
    per-s clamped masks (keys j<0); k^T is zero-padded on the left so
    all score matmuls use one uniform 144-wide window.
  * exp: one scalar-engine activation per group (PSUM->SBUF fp16)
    via a 4-dim AP that skips the bank pad, writing into a 145-col
    slot pitch whose last column is pre-filled with exp(sink).
  * row sums: one segmented DVE tensor_reduce per group over
    [128, slots, 145] -- each query's full 128-wide band lies in one
    144-col slot, and the sink column seeds the sum, so the reduce
    directly yields the softmax denominator.  One batched reciprocal.
  * normalize: one wide scalar_tensor_tensor per group (rec broadcast
    along each slot via a 0-stride AP); three groups run per-slot
    Copy-with-scale on the scalar engine instead to balance DVE/ACT.
  * all DRAM<->SBUF traffic uses per-chunk contiguous tensors (column
    slices of a big row-major tensor DMA as strided 4KB descriptors
    at ~1/8th bandwidth); inputs are split across the sync/scalar
    HWDGE queues + the slower gpsimd software queue by need-time.
  * output: 11 group-major [128, 864] fp16 strips; host scatters the
    diagonal band into the zero-filled [M, T, T] fp32 result.
"""

import math

import numpy as np

T = 1024
HKV = 8
M = 8
D = 64
WINDOW = 128
SM_SCALE = 1.0 / math.sqrt(D)

B = 16                    # queries per sub-block
NS = T // B               # 64 sub-blocks
WIN = B + WINDOW          # 144 cols per sub-block window
GROUP = 6                 # sub-blocks per PSUM tile (2 banks, 3 slots each)
ACT_MUL_GROUPS = (1, 4, 9)  # groups whose normalize runs on the scalar engine
QCH = ((0, 8), (8, 24), (24, 44), (44, 64))  # q DMA chunks (slot ranges)
NG = (NS + GROUP - 1) // GROUP  # 11 groups (last has 4)
KPAD = WINDOW             # zero pad at the left of k^T
MASKVAL = -10000.0
NMASK = 4                 # bank-tiles: [s012], [s345], [s67|reg], [reg x3]
BANKW = 3 * WIN           # 432 mask cols per bank

_PROGRAM = None


def _slot_col(j):
    """PSUM col offset of slot j (0..5) within a [128, 1024] 2-bank tile."""
    return 512 * (j // 3) + 144 * (j % 3)


def _build_program():
    import concourse.bacc as bacc
    import concourse.bass as bass
    import concourse.tile as tile
    from concourse import mybir

    f32 = mybir.dt.float32
    f16 = mybir.dt.float16
    bf16 = mybir.dt.bfloat16
    Exp = mybir.ActivationFunctionType.Exp
    Copy = mybir.ActivationFunctionType.Copy
    Alu = mybir.AluOpType

    nc = bacc.Bacc("TRN2")
    # stationary q: rows 0..63 = bf16(q*scale), 64..127 = bf16 residual;
    # free dim: sub-blocks x 128 (p = m*16 + r16).  Chunked into separate
    # contiguous DRAM tensors: a column slice of one big tensor would DMA
    # as 128 strided 4KB descriptors at ~1/8th bandwidth.
    qcs = [
        nc.dram_tensor(
            f"qc{i}", [2 * D, (b - a) * 128], bf16, kind="ExternalInput"
        )
        for i, (a, b) in enumerate(QCH)
    ]
    # constant inputs in three priority-ordered contiguous tensors
    # (a column slice of one big tensor would DMA as strided rows):
    #   cA = [ident 128 | mask bank A 432 | mask bank B 432]
    #   cB = [kT 1280 | esink 2]   (kT: k^T dup halves, 128 zero-pad left;
    #                               esink bitcast to 2 bf16 cols)
    #   cC = [mask bank C 432 | mask bank R 432]
    cA1 = nc.dram_tensor("cA1", [128, 128 + BANKW], bf16, kind="ExternalInput")
    cA2 = nc.dram_tensor("cA2", [128, BANKW], bf16, kind="ExternalInput")
    cB = nc.dram_tensor("cB", [128, KPAD + T + 2], bf16, kind="ExternalInput")
    cC = nc.dram_tensor("cC", [128, 2 * BANKW], bf16, kind="ExternalInput")
    outb = nc.dram_tensor("outb", [NG, 128, GROUP * WIN], f16, kind="ExternalOutput")

    with tile.TileContext(nc) as tc:
        with (
            tc.tile_pool(name="singles", bufs=1) as singles,
            tc.tile_pool(name="psum", bufs=4, space="PSUM") as psum_pool,
            tc.tile_pool(name="pout", bufs=4) as pout,
            tc.tile_pool(name="stats", bufs=6) as stats,
        ):
            NA = 128 + 2 * BANKW
            NB = KPAD + T + 2
            const_sb = singles.tile([128, NA + NB + 2 * BANKW], bf16)
            q_sb = singles.tile([2 * D, NS * 128], bf16)

            id_sb = const_sb[:, 0:128]
            kT_sb = const_sb[:, NA : NA + KPAD + T]
            esink_sb = const_sb[:, NA + KPAD + T : NA + NB].bitcast(f32)
            # mask bank tile t -> sbuf col offset
            mask_off = [128, 128 + BANKW, NA + NB, NA + NB + BANKW]

            # consts on the sync HWDGE queue in 3 priority chunks (the
            # first mask matmuls need id+banks A/B; scores then need kT;
            # banks C/R + esink are needed a few groups later) + last q
            # chunk; first three q chunks on the scalar HWDGE queue (the
            # gpsimd software-DGE path is ~3x slower -- outputs only)
            nc.sync.dma_start(
                out=const_sb[:, 0 : 128 + BANKW], in_=cA1[:]
            )
            nc.sync.dma_start(out=const_sb[:, NA : NA + NB], in_=cB[:])
            nc.sync.dma_start(out=const_sb[:, 128 + BANKW : NA], in_=cA2[:])
            # banks C/R are needed by group 1 already -- the gpsimd queue
            # is otherwise idle at startup, so despite its ~3x slower
            # software path they land in time
            nc.gpsimd.dma_start(out=const_sb[:, NA + NB :], in_=cC[:])
            for i, (a, b) in enumerate(QCH):
                eng = nc.sync if i == len(QCH) - 1 else nc.scalar
                eng.dma_start(
                    out=q_sb[:, a * 128 : b * 128], in_=qcs[i][:]
                )

            # dedicated out tile for the ragged last group: its unwritten
            # tail cols are zeroed once so the full-width DMA reads no
            # stale rotating-pool data
            out_last = singles.tile([128, GROUP * WIN], f16)
            nc.vector.memset(out_last[:, (NS - (NG - 1) * GROUP) * WIN :], 0)

            # E tiles rotate manually so each tile's per-slot sink column
            # (col 144 of the 145-col slot pitch) is filled exactly once:
            # the row-sum reduce then covers it, yielding den = sum + esink
            # with no separate add
            E_tiles = []
            for i in range(4):
                Et = singles.tile(
                    [128, GROUP * (WIN + 1)], f16, name=f"Etile{i}"
                )
                E_tiles.append(Et)
                nc.vector.tensor_scalar(
                    out=Et[:].rearrange("p (s n) -> p s n", n=WIN + 1)[
                        :, :, WIN : WIN + 1
                    ],
                    in0=esink_sb[:, 0:1]
                    .unsqueeze(1)
                    .broadcast_to((128, GROUP, 1)),
                    scalar1=1.0,
                    scalar2=None,
                    op0=Alu.mult,
                )

            pending = None  # deferred ACT-group normalize emitter
            for g in range(NG):
                s0 = g * GROUP
                nslot = min(GROUP, NS - s0)
                ps = psum_pool.tile([128, 1024], f32, name="ps", tag="ps")
                # one wide mask-bias matmul per PSUM bank opens the
                # bank's accumulation group (identity stationary)
                # per bank: one wide mask-bias matmul opens the bank's
                # accumulation group (identity stationary), then the
                # bank's score matmuls follow immediately -- bank 1's
                # mask tile may still be in flight while bank 0 computes
                for bank in range(2):
                    nb = min(3, nslot - 3 * bank)
                    if nb <= 0:
                        break
                    gb = 2 * g + bank  # global bank index
                    t = gb if gb < 3 else 3
                    mo = mask_off[t]
                    nc.tensor.matmul(
                        ps[:, 512 * bank : 512 * bank + nb * WIN],
                        id_sb,
                        const_sb[:, mo : mo + nb * WIN],
                        start=True,
                        stop=False,
                    )
                    # score matmuls: stationary = [q_hi; q_lo] per
                    # sub-block, moving = [k; k] window (zero-padded left
                    # edge); only the bank's last slot carries stop (sim
                    # group tracking is per 2KB bank)
                    for jj in range(nb):
                        j = 3 * bank + jj
                        s = s0 + j
                        c = _slot_col(j)
                        last_in_bank = (jj == nb - 1)
                        nc.tensor.matmul(
                            ps[:, c : c + WIN],
                            q_sb[:, s * 128 : (s + 1) * 128],
                            kT_sb[:, B * s : B * s + WIN],
                            start=False,
                            stop=last_in_bank,
                        )
                # exp (PSUM -> SBUF fp16) into the 145-pitch E tile,
                # skipping the 80-col bank pad and the sink columns
                ncols = nslot * WIN
                E = E_tiles[g % 4]
                E5 = E[:].rearrange("p (s n) -> p s n", n=WIN + 1)
                if nslot == GROUP:
                    ps4 = ps[:].rearrange("p (b n) -> p b n", n=512)[
                        :, :, 0:432
                    ].rearrange("p b (s n) -> p b s n", n=WIN)
                    E4 = E[:].rearrange(
                        "p (b s n) -> p b s n", s=3, n=WIN + 1
                    )[:, :, :, 0:WIN]
                    nc.scalar.activation(out=E4, in_=ps4, func=Exp)
                else:
                    nc.scalar.activation(
                        out=E5[:, 0:3, 0:WIN],
                        in_=ps[:, 0:432].rearrange("p (s n) -> p s n", n=WIN),
                        func=Exp,
                    )
                    nc.scalar.activation(
                        out=E5[:, 3:4, 0:WIN],
                        in_=ps[:, 512 : 512 + WIN].rearrange(
                            "p (s n) -> p s n", n=WIN
                        ),
                        func=Exp,
                    )

                # flush the previous ACT-mul group's deferred normalize
                # now that this group's exp is already in the ACT stream
                if pending is not None:
                    pending()
                    pending = None

                # batched row sums over [slot cols + sink col]: each slot's
                # 144 cols are the query's complete valid band, so the sum
                # (seeded with esink via col 144) IS the denominator
                den = stats.tile([128, GROUP], f32)
                nc.vector.tensor_reduce(
                    out=den[:, 0:nslot],
                    in_=E5[:, 0:nslot, :],
                    axis=mybir.AxisListType.X,
                    op=Alu.add,
                )
                rec = stats.tile([128, GROUP], f32)
                nc.vector.reciprocal(rec[:, 0:nslot], den[:, 0:nslot])

                # normalize: out = E * rec.  DVE groups use one wide
                # scalar_tensor_tensor (rec broadcast via 0-stride AP);
                # ACT groups use per-slot Copy-with-scale activations to
                # offload the DVE.  ACT-group muls wait on this group's
                # reciprocal (DVE) -- emitting them before the NEXT
                # group's exp would head-of-line block it in the ACT
                # engine's strict-FIFO stream, so they are deferred one
                # iteration (see the `pending` flush above).
                def _normalize(g=g, nslot=nslot, ncols=ncols, E5=E5, rec=rec):
                    out_sb = (
                        out_last
                        if nslot < GROUP
                        else pout.tile([128, GROUP * WIN], f16, name="out_sb")
                    )
                    if g in ACT_MUL_GROUPS:
                        for j in range(nslot):
                            nc.scalar.activation(
                                out=out_sb[:, j * WIN : (j + 1) * WIN],
                                in_=E5[:, j, 0:WIN],
                                func=Copy,
                                scale=rec[:, j : j + 1],
                            )
                    else:
                        nc.vector.scalar_tensor_tensor(
                            out=out_sb[:, 0:ncols].rearrange(
                                "p (s n) -> p s n", n=WIN
                            ),
                            in0=E5[:, 0:nslot, 0:WIN],
                            scalar=1.0,
                            in1=rec[:, 0:nslot].unsqueeze(-1).broadcast_to(
                                (128, nslot, WIN)
                            ),
                            op0=Alu.mult,
                            op1=Alu.mult,
                        )
                    eng = nc.sync if g % 2 == 0 else nc.gpsimd
                    eng.dma_start(out=outb[g], in_=out_sb[:])

                if g in ACT_MUL_GROUPS:
                    pending = _normalize
                else:
                    _normalize()

            if pending is not None:
                pending()

    nc.compile()
    return nc


def _get_program():
    global _PROGRAM
    if _PROGRAM is None:
        _PROGRAM = _build_program()
    return _PROGRAM


def _build_masks():
    """[128, 4*432] bf16 bank-tiles: [s0|s1|s2], [s3|s4|s5], [s6|s7|reg],
    [reg|reg|reg].  valid(s, r16, c): c > r16, c <= r16+128, and (for
    clamped s<8) c >= 128-16s."""
    import ml_dtypes

    r16 = (np.arange(128) % 16)[:, None]
    c = np.arange(WIN)[None, :]
    reg = (c > r16) & (c <= r16 + WINDOW)

    def slot(s):
        v = reg & (c >= (WINDOW - B * s)) if s < 8 else reg
        return np.where(v, 0.0, MASKVAL)

    banks = []
    for t in range(NMASK):
        ss = [3 * t, 3 * t + 1, 3 * t + 2] if t < 3 else [8, 8, 8]
        banks.append(np.concatenate([slot(s) for s in ss], axis=1))
    return np.concatenate(banks, axis=1).astype(ml_dtypes.bfloat16)


def _make_in_maps(q, k, sinks):
    import ml_dtypes

    bf = ml_dtypes.bfloat16
    q = np.asarray(q, dtype=np.float32)
    k = np.asarray(k, dtype=np.float32)
    sinks = np.asarray(sinks, dtype=np.float32)
    maskt = _build_masks()
    ident = np.eye(128, dtype=np.float32).astype(bf)
    esink_hm = np.exp(sinks.reshape(HKV, M))
    in_maps = []
    for h in range(HKV):
        # stationary q: [2D, NS*128]; col index = s*128 + m*16 + r16
        qs = (q[:, h] * SM_SCALE).astype(np.float32)  # [T, M, D]
        qs = qs.reshape(NS, B, M, D).transpose(3, 0, 2, 1)  # [D, NS, M, B]
        qh = qs.astype(bf)
        ql = (qs - qh.astype(np.float32)).astype(bf)
        qst = np.concatenate([qh, ql], axis=0).reshape(2 * D, NS * 128)
        qchunks = {
            f"qc{i}": np.ascontiguousarray(qst[:, a * 128 : b * 128])
            for i, (a, b) in enumerate(QCH)
        }
        # moving k^T: [2D, 128+T], zero left pad, duplicated halves
        kh = k[:, h].transpose(1, 0).astype(bf)  # [D, T]
        kp = np.zeros((2 * D, KPAD + T), dtype=bf)
        kp[0:D, KPAD:] = kh
        kp[D:, KPAD:] = kh
        # esink per partition p = m*16 + r16, bitcast fp32 -> 2 bf16 cols
        esinkc = np.repeat(esink_hm[h], B).reshape(128, 1).astype(np.float32)
        esink2 = esinkc.view(np.uint16).view(bf)  # [128, 2]
        bankw = 3 * WIN
        mA, mB, mC, mR = (maskt[:, i * bankw : (i + 1) * bankw] for i in range(4))
        in_maps.append(
            {
                "cA1": np.ascontiguousarray(np.concatenate([ident, mA], axis=1)),
                "cA2": np.ascontiguousarray(mB),
                "cB": np.ascontiguousarray(np.concatenate([kp, esink2], axis=1)),
                "cC": np.ascontiguousarray(np.concatenate([mC, mR], axis=1)),
                **qchunks,
            }
        )
    return in_maps


def _assemble(outb_all):
    """outb_all: [nh, NG, 128, GROUP*WIN] fp16 device strips -> full
    [nh, M, T, T] fp32 probs (zeros outside the band)."""
    ob = np.asarray(outb_all).astype(np.float32)
    nh = ob.shape[0]
    # [nh, g, p, j, c] -> [nh, m, r16, s, c]
    v5 = ob.reshape(nh, NG, M, B, GROUP, WIN)
    v = np.empty((nh, M, B, NS, WIN), dtype=np.float32)
    for g in range(NG):
        n = min(GROUP, NS - g * GROUP)
        v[:, :, :, g * GROUP : g * GROUP + n] = v5[:, g, :, :, :n]
    full = np.zeros((nh, M, T, T), dtype=np.float32)
    for s in range(NS):
        j0 = B * s - WINDOW
        if s < 8:
            full[:, :, B * s : B * s + B, 0 : B * s + B] = v[
                :, :, :, s, WINDOW - B * s :
            ]
        else:
            full[:, :, B * s : B * s + B, j0 : j0 + WIN] = v[:, :, :, s, :]
    return full


def _run(q, k, sinks, trace=False):
    from concourse.bass_utils import run_bass_kernel_spmd

    nc = _get_program()
    in_maps = _make_in_maps(q, k, sinks)
    res = run_bass_kernel_spmd(nc, in_maps, list(range(HKV)), trace=trace)
    outb_all = np.stack([r["outb"] for r in res.results], axis=0)
    return _assemble(outb_all), res


def kernel(q, k, sinks):
    out, _ = _run(q, k, sinks, trace=False)
    return out


# revision 37
# speedup vs baseline: 1.1202x; 1.1202x over previous
"""GPT-OSS attention QK+softmax block (sliding-window 128, softmax with sink)
for Trainium2, sharded over the 8 kv heads across 8 NeuronCores.

Reference computation (per kv head h, per q-head m):
    S = (q[:, h, m] @ k[:, h].T) / sqrt(64)            # [T, T]
    S += causal & sliding-window(128) mask             # band of width 128
    probs = softmax([S, sink_{h,m}])[..., :-1]         # sink column dropped

Device kernel structure (per core = one kv head), v2 "stacked" layout:
  * the 128 PSUM partitions hold 8 q-heads x 16 queries (p = m*16 + r16),
    so one matmul covers ALL m-heads for a 16-query sub-block s
    (queries 16s..16s+15).  The key window for those queries is only
    16 + 128 = 144 wide (cols = keys 16s-128 .. 16s+16), vs 256 in a
    128-query blocking -- every downstream stage (exp, row-sums,
    normalize, output DMA) shrinks ~40%.
  * 64 sub-blocks, processed in 11 groups of 6 (last group 4).  One
    PSUM tile per group = 2 banks; each bank holds 3 slots of 144
    fp32 cols (+80 pad).
  * matmul cost on the PE scales with OUTPUT cols only, so q enters
    exactly for free: stationary = [q_hi; q_lo] (128 contraction rows),
    moving = [k_bf16; k_bf16].  k is single-rounded (max rel err
    ~1.1e-2 incl fp16 storage, vs the 2e-2 gate).
  * causal/sliding-window mask folded into scores on the PE: identity-
    weight matmul accumulates a {0,-1e4} bias per slot so exp
    underflows masked entries to exactly 0.  Sub-blocks s<8 use
    per-s clamped masks (keys j<0); k^T is zero-padded on the left so
    all score matmuls use one uniform 144-wide window.
  * exp: one scalar-engine activation per group (PSUM->SBUF fp16)
    via a 4-dim AP that skips the bank pad, writing into a 145-col
    slot pitch whose last column is pre-filled with exp(sink).
  * row sums: one segmented DVE tensor_reduce per group over
    [128, slots, 145] -- each query's full 128-wide band lies in one
    144-col slot, and the sink column seeds the sum, so the reduce
    directly yields the softmax denominator.  One batched reciprocal.
  * normalize: one wide scalar_tensor_tensor per group (rec broadcast
    along each slot via a 0-stride AP); three groups run per-slot
    Copy-with-scale on the scalar engine instead to balance DVE/ACT.
  * all DRAM<->SBUF traffic uses per-chunk contiguous tensors (column
    slices of a big row-major tensor DMA as strided 4KB descriptors
    at ~1/8th bandwidth); inputs are split across the sync/scalar
    HWDGE queues + the slower gpsimd software queue by need-time.
  * output: 11 group-major [128, 864] fp16 strips; host scatters the
    diagonal band into the zero-filled [M, T, T] fp32 result.
"""

import math

import numpy as np

T = 1024
HKV = 8
M = 8
D = 64
WINDOW = 128
SM_SCALE = 1.0 / math.sqrt(D)

B = 16                    # queries per sub-block
NS = T // B               # 64 sub-blocks
WIN = B + WINDOW          # 144 cols per sub-block window
GROUP = 6                 # sub-blocks per PSUM tile (2 banks, 3 slots each)
ACT_MUL_GROUPS = (1, 4, 9)  # groups whose normalize runs on the scalar engine
QCH = ((0, 8), (8, 24), (24, 44), (44, 64))  # q DMA chunks (slot ranges)
NG = (NS + GROUP - 1) // GROUP  # 11 groups (last has 4)
KPAD = WINDOW             # zero pad at the left of k^T
MASKVAL = -10000.0
NMASK = 4                 # bank-tiles: [s012], [s345], [s67|reg], [reg x3]
BANKW = 3 * WIN           # 432 mask cols per bank

_PROGRAM = None


def _slot_col(j):
    """PSUM col offset of slot j (0..5) within a [128, 1024] 2-bank tile."""
    return 512 * (j // 3) + 144 * (j % 3)


def _build_program():
    import concourse.bacc as bacc
    import concourse.bass as bass
    import concourse.tile as tile
    from concourse import mybir

    f32 = mybir.dt.float32
    f16 = mybir.dt.float16
    bf16 = mybir.dt.bfloat16
    Exp = mybir.ActivationFunctionType.Exp
    Copy = mybir.ActivationFunctionType.Copy
    Alu = mybir.AluOpType

    nc = bacc.Bacc("TRN2")
    # stationary q: rows 0..63 = bf16(q*scale), 64..127 = bf16 residual;
    # free dim: sub-blocks x 128 (p = m*16 + r16).  Chunked into separate
    # contiguous DRAM tensors: a column slice of one big tensor would DMA
    # as 128 strided 4KB descriptors at ~1/8th bandwidth.
    qcs = [
        nc.dram_tensor(
            f"qc{i}", [2 * D, (b - a) * 128], bf16, kind="ExternalInput"
        )
        for i, (a, b) in enumerate(QCH)
    ]
    # constant inputs in three priority-ordered contiguous tensors
    # (a column slice of one big tensor would DMA as strided rows):
    #   cA = [ident 128 | mask bank A 432 | mask bank B 432]
    #   cB = [kT 1280 | esink 2]   (kT: k^T dup halves, 128 zero-pad left;
    #                               esink bitcast to 2 bf16 cols)
    #   cC = [mask bank C 432 | mask bank R 432]
    cA = nc.dram_tensor("cA", [128, 128 + 2 * BANKW], bf16, kind="ExternalInput")
    cB = nc.dram_tensor("cB", [128, KPAD + T + 2], bf16, kind="ExternalInput")
    cC = nc.dram_tensor("cC", [128, 2 * BANKW], bf16, kind="ExternalInput")
    outb = nc.dram_tensor("outb", [NG, 128, GROUP * WIN], f16, kind="ExternalOutput")

    with tile.TileContext(nc) as tc:
        with (
            tc.tile_pool(name="singles", bufs=1) as singles,
            tc.tile_pool(name="psum", bufs=4, space="PSUM") as psum_pool,
            tc.tile_pool(name="pout", bufs=4) as pout,
            tc.tile_pool(name="stats", bufs=6) as stats,
        ):
            NA = 128 + 2 * BANKW
            NB = KPAD + T + 2
            const_sb = singles.tile([128, NA + NB + 2 * BANKW], bf16)
            q_sb = singles.tile([2 * D, NS * 128], bf16)

            id_sb = const_sb[:, 0:128]
            kT_sb = const_sb[:, NA : NA + KPAD + T]
            esink_sb = const_sb[:, NA + KPAD + T : NA + NB].bitcast(f32)
            # mask bank tile t -> sbuf col offset
            mask_off = [128, 128 + BANKW, NA + NB, NA + NB + BANKW]

            # consts on the sync HWDGE queue in 3 priority chunks (the
            # first mask matmuls need id+banks A/B; scores then need kT;
            # banks C/R + esink are needed a few groups later) + last q
            # chunk; first three q chunks on the scalar HWDGE queue (the
            # gpsimd software-DGE path is ~3x slower -- outputs only)
            nc.sync.dma_start(out=const_sb[:, 0:NA], in_=cA[:])
            nc.sync.dma_start(out=const_sb[:, NA : NA + NB], in_=cB[:])
            # banks C/R are needed by group 1 already -- the gpsimd queue
            # is otherwise idle at startup, so despite its ~3x slower
            # software path they land in time
            nc.gpsimd.dma_start(out=const_sb[:, NA + NB :], in_=cC[:])
            for i, (a, b) in enumerate(QCH):
                eng = nc.sync if i == len(QCH) - 1 else nc.scalar
                eng.dma_start(
                    out=q_sb[:, a * 128 : b * 128], in_=qcs[i][:]
                )

            # dedicated out tile for the ragged last group: its unwritten
            # tail cols are zeroed once so the full-width DMA reads no
            # stale rotating-pool data
            out_last = singles.tile([128, GROUP * WIN], f16)
            nc.vector.memset(out_last[:, (NS - (NG - 1) * GROUP) * WIN :], 0)

            # E tiles rotate manually so each tile's per-slot sink column
            # (col 144 of the 145-col slot pitch) is filled exactly once:
            # the row-sum reduce then covers it, yielding den = sum + esink
            # with no separate add
            E_tiles = []
            for i in range(4):
                Et = singles.tile(
                    [128, GROUP * (WIN + 1)], f16, name=f"Etile{i}"
                )
                E_tiles.append(Et)
                nc.vector.tensor_scalar(
                    out=Et[:].rearrange("p (s n) -> p s n", n=WIN + 1)[
                        :, :, WIN : WIN + 1
                    ],
                    in0=esink_sb[:, 0:1]
                    .unsqueeze(1)
                    .broadcast_to((128, GROUP, 1)),
                    scalar1=1.0,
                    scalar2=None,
                    op0=Alu.mult,
                )

            pending = None  # deferred ACT-group normalize emitter
            for g in range(NG):
                s0 = g * GROUP
                nslot = min(GROUP, NS - s0)
                ps = psum_pool.tile([128, 1024], f32, name="ps", tag="ps")
                # one wide mask-bias matmul per PSUM bank opens the
                # bank's accumulation group (identity stationary)
                # one wide mask-bias matmul per PSUM bank opens the
                # bank's accumulation group (identity stationary)
                for bank in range(2):
                    nb = min(3, nslot - 3 * bank)
                    if nb <= 0:
                        break
                    gb = 2 * g + bank  # global bank index
                    t = gb if gb < 3 else 3
                    mo = mask_off[t]
                    nc.tensor.matmul(
                        ps[:, 512 * bank : 512 * bank + nb * WIN],
                        id_sb,
                        const_sb[:, mo : mo + nb * WIN],
                        start=True,
                        stop=False,
                    )
                # score matmuls: stationary = [q_hi; q_lo] per sub-block,
                # moving = [k; k] window (zero-padded left edge); only
                # the last slot of each bank carries stop (sim group
                # tracking is per 2KB bank)
                for j in range(nslot):
                    s = s0 + j
                    c = _slot_col(j)
                    last_in_bank = (j % 3 == 2) or (j == nslot - 1)
                    nc.tensor.matmul(
                        ps[:, c : c + WIN],
                        q_sb[:, s * 128 : (s + 1) * 128],
                        kT_sb[:, B * s : B * s + WIN],
                        start=False,
                        stop=last_in_bank,
                    )
                # exp (PSUM -> SBUF fp16) into the 145-pitch E tile,
                # skipping the 80-col bank pad and the sink columns
                ncols = nslot * WIN
                E = E_tiles[g % 4]
                E5 = E[:].rearrange("p (s n) -> p s n", n=WIN + 1)
                if nslot == GROUP:
                    ps4 = ps[:].rearrange("p (b n) -> p b n", n=512)[
                        :, :, 0:432
                    ].rearrange("p b (s n) -> p b s n", n=WIN)
                    E4 = E[:].rearrange(
                        "p (b s n) -> p b s n", s=3, n=WIN + 1
                    )[:, :, :, 0:WIN]
                    nc.scalar.activation(out=E4, in_=ps4, func=Exp)
                else:
                    nc.scalar.activation(
                        out=E5[:, 0:3, 0:WIN],
                        in_=ps[:, 0:432].rearrange("p (s n) -> p s n", n=WIN),
                        func=Exp,
                    )
                    nc.scalar.activation(
                        out=E5[:, 3:4, 0:WIN],
                        in_=ps[:, 512 : 512 + WIN].rearrange(
                            "p (s n) -> p s n", n=WIN
                        ),
                        func=Exp,
                    )

                # flush the previous ACT-mul group's deferred normalize
                # now that this group's exp is already in the ACT stream
                if pending is not None:
                    pending()
                    pending = None

                # batched row sums over [slot cols + sink col]: each slot's
                # 144 cols are the query's complete valid band, so the sum
                # (seeded with esink via col 144) IS the denominator
                den = stats.tile([128, GROUP], f32)
                nc.vector.tensor_reduce(
                    out=den[:, 0:nslot],
                    in_=E5[:, 0:nslot, :],
                    axis=mybir.AxisListType.X,
                    op=Alu.add,
                )
                rec = stats.tile([128, GROUP], f32)
                nc.vector.reciprocal(rec[:, 0:nslot], den[:, 0:nslot])

                # normalize: out = E * rec.  DVE groups use one wide
                # scalar_tensor_tensor (rec broadcast via 0-stride AP);
                # ACT groups use per-slot Copy-with-scale activations to
                # offload the DVE.  ACT-group muls wait on this group's
                # reciprocal (DVE) -- emitting them before the NEXT
                # group's exp would head-of-line block it in the ACT
                # engine's strict-FIFO stream, so they are deferred one
                # iteration (see the `pending` flush above).
                def _normalize(g=g, nslot=nslot, ncols=ncols, E5=E5, rec=rec):
                    out_sb = (
                        out_last
                        if nslot < GROUP
                        else pout.tile([128, GROUP * WIN], f16, name="out_sb")
                    )
                    if g in ACT_MUL_GROUPS:
                        for j in range(nslot):
                            nc.scalar.activation(
                                out=out_sb[:, j * WIN : (j + 1) * WIN],
                                in_=E5[:, j, 0:WIN],
                                func=Copy,
                                scale=rec[:, j : j + 1],
                            )
                    else:
                        nc.vector.scalar_tensor_tensor(
                            out=out_sb[:, 0:ncols].rearrange(
                                "p (s n) -> p s n", n=WIN
                            ),
                            in0=E5[:, 0:nslot, 0:WIN],
                            scalar=1.0,
                            in1=rec[:, 0:nslot].unsqueeze(-1).broadcast_to(
                                (128, nslot, WIN)
                            ),
                            op0=Alu.mult,
                            op1=Alu.mult,
                        )
                    eng = nc.sync if g % 2 == 0 else nc.gpsimd
                    eng.dma_start(out=outb[g], in_=out_sb[:])

                if g in ACT_MUL_GROUPS:
                    pending = _normalize
                else:
                    _normalize()

            if pending is not None:
                pending()

    nc.compile()
    return nc


def _get_program():
    global _PROGRAM
    if _PROGRAM is None:
        _PROGRAM = _build_program()
    return _PROGRAM


def _build_masks():
    """[128, 4*432] bf16 bank-tiles: [s0|s1|s2], [s3|s4|s5], [s6|s7|reg],
    [reg|reg|reg].  valid(s, r16, c): c > r16, c <= r16+128, and (for
    clamped s<8) c >= 128-16s."""
    import ml_dtypes

    r16 = (np.arange(128) % 16)[:, None]
    c = np.arange(WIN)[None, :]
    reg = (c > r16) & (c <= r16 + WINDOW)

    def slot(s):
        v = reg & (c >= (WINDOW - B * s)) if s < 8 else reg
        return np.where(v, 0.0, MASKVAL)

    banks = []
    for t in range(NMASK):
        ss = [3 * t, 3 * t + 1, 3 * t + 2] if t < 3 else [8, 8, 8]
        banks.append(np.concatenate([slot(s) for s in ss], axis=1))
    return np.concatenate(banks, axis=1).astype(ml_dtypes.bfloat16)


def _make_in_maps(q, k, sinks):
    import ml_dtypes

    bf = ml_dtypes.bfloat16
    q = np.asarray(q, dtype=np.float32)
    k = np.asarray(k, dtype=np.float32)
    sinks = np.asarray(sinks, dtype=np.float32)
    maskt = _build_masks()
    ident = np.eye(128, dtype=np.float32).astype(bf)
    esink_hm = np.exp(sinks.reshape(HKV, M))
    in_maps = []
    for h in range(HKV):
        # stationary q: [2D, NS*128]; col index = s*128 + m*16 + r16
        qs = (q[:, h] * SM_SCALE).astype(np.float32)  # [T, M, D]
        qs = qs.reshape(NS, B, M, D).transpose(3, 0, 2, 1)  # [D, NS, M, B]
        qh = qs.astype(bf)
        ql = (qs - qh.astype(np.float32)).astype(bf)
        qst = np.concatenate([qh, ql], axis=0).reshape(2 * D, NS * 128)
        qchunks = {
            f"qc{i}": np.ascontiguousarray(qst[:, a * 128 : b * 128])
            for i, (a, b) in enumerate(QCH)
        }
        # moving k^T: [2D, 128+T], zero left pad, duplicated halves
        kh = k[:, h].transpose(1, 0).astype(bf)  # [D, T]
        kp = np.zeros((2 * D, KPAD + T), dtype=bf)
        kp[0:D, KPAD:] = kh
        kp[D:, KPAD:] = kh
        # esink per partition p = m*16 + r16, bitcast fp32 -> 2 bf16 cols
        esinkc = np.repeat(esink_hm[h], B).reshape(128, 1).astype(np.float32)
        esink2 = esinkc.view(np.uint16).view(bf)  # [128, 2]
        bankw = 3 * WIN
        mA, mB, mC, mR = (maskt[:, i * bankw : (i + 1) * bankw] for i in range(4))
        in_maps.append(
            {
                "cA": np.ascontiguousarray(np.concatenate([ident, mA, mB], axis=1)),
                "cB": np.ascontiguousarray(np.concatenate([kp, esink2], axis=1)),
                "cC": np.ascontiguousarray(np.concatenate([mC, mR], axis=1)),
                **qchunks,
            }
        )
    return in_maps


def _assemble(outb_all):
    """outb_all: [nh, NG, 128, GROUP*WIN] fp16 device strips -> full
    [nh, M, T, T] fp32 probs (zeros outside the band)."""
    ob = np.asarray(outb_all).astype(np.float32)
    nh = ob.shape[0]
    # [nh, g, p, j, c] -> [nh, m, r16, s, c]
    v5 = ob.reshape(nh, NG, M, B, GROUP, WIN)
    v = np.empty((nh, M, B, NS, WIN), dtype=np.float32)
    for g in range(NG):
        n = min(GROUP, NS - g * GROUP)
        v[:, :, :, g * GROUP : g * GROUP + n] = v5[:, g, :, :, :n]
    full = np.zeros((nh, M, T, T), dtype=np.float32)
    for s in range(NS):
        j0 = B * s - WINDOW
        if s < 8:
            full[:, :, B * s : B * s + B, 0 : B * s + B] = v[
                :, :, :, s, WINDOW - B * s :
            ]
        else:
            full[:, :, B * s : B * s + B, j0 : j0 + WIN] = v[:, :, :, s, :]
    return full


def _run(q, k, sinks, trace=False):
    from concourse.bass_utils import run_bass_kernel_spmd

    nc = _get_program()
    in_maps = _make_in_maps(q, k, sinks)
    res = run_bass_kernel_spmd(nc, in_maps, list(range(HKV)), trace=trace)
    outb_all = np.stack([r["outb"] for r in res.results], axis=0)
    return _assemble(outb_all), res


def kernel(q, k, sinks):
    out, _ = _run(q, k, sinks, trace=False)
    return out
